# revision 1
# baseline (speedup 1.0000x reference)
"""Trainium2 Bass kernel for nn_DualAddressingPhasor.

Math: the phasor cumsum-bind/retrieve is causal linear attention:
  retrieved[l] = sum_{l'<=l} (sum_k cos(phi_l,k - phi_l',k)) * value[l']
Per 512-row chunk this is (1) a carried state [2K, D] = CS^T @ value over
the prefix plus (2) intra-chunk attention triu(Cc@Cc^T + Sc@Sc^T) @ value_c.

Sharding: 8 cores = 2 batches x 4 sequence chunks of 512. Uniform SPMD
program; per-core variation is entirely in the data (right-aligned
zero-padded prefix, host-precomputed positional phase tables with zero
signs in the padding so padded rows contribute nothing).

Perf notes (vs the fp32r v1 at ~86us, now ~63-71us):
- all matmul operands bf16: fp32r matmuls self-load weights (~+70ns per
  512-row matmul); bf16 hits the 216ns peak with LDWEIGHTS overlapped.
- a dozen warmup matmuls on zeros ramp the PE clock out of its low
  pstate while the first DMAs land.
- per-chunk two-phase (k01/k23) accumulation in (a) consumes the
  k-split chunk-0 DMAs as they land; all x chunks are split across the
  sync/gpsimd queues (per-queue DMA bandwidth is only ~83 GB/s).
- the act-table RAM holds ONE function set at a time and the Tile
  scheduler freely interleaves ready work, so sins are token-gated
  behind the last tanh (one tanh->sin table switch instead of six) and
  the sqrt table is preloaded right after the sins, off-critical-path.
- natural-layout prefix phases come from 12 tiny per-chunk transposes
  of the content tile; chunk 3 (own chunk) only needs the freq-major
  path. LayerNorm stats are accumulated by matmul interleaved with the
  retrieve matmuls, transposed to column space via 8 tiny PE
  transposes, and the rstd math runs on [128,4] tiles while the output
  matmuls proceed.
"""
import sys

for _p in ("/opt/trn_rl_repo",):
    if _p not in sys.path:
        sys.path.append(_p)

import numpy as np
import ml_dtypes

import concourse.bacc as bacc
import concourse.tile as tile
import concourse.mybir as mybir
from concourse.bass import ts
from concourse.bass_utils import run_bass_kernel_spmd
from concourse.masks import make_identity

F32 = mybir.dt.float32
BF16 = mybir.dt.bfloat16
AF = mybir.ActivationFunctionType
ALU = mybir.AluOpType

D = 512
K = 32
B = 2
L = 2048
CH = 512          # chunk rows per core
T = 2048          # padded rows processed per core
NCORE = 8

_NC_CACHE = {}
LAST_RESULT = None
RUN_KWARGS = {}


def _build(zero_bv: bool, kc_val: float):
    nc = bacc.Bacc("TRN2", num_devices=NCORE)

    xt = nc.dram_tensor("xt", [4, 128, 4, CH], BF16, kind="ExternalInput")
    w1f = nc.dram_tensor("w1f", [128, 1, D], BF16, kind="ExternalInput")
    w1 = nc.dram_tensor("w1", [128, 4, D], BF16, kind="ExternalInput")
    w2 = nc.dram_tensor("w2", [128, 4, K], BF16, kind="ExternalInput")
    wv = nc.dram_tensor("wv", [128, 4, D], BF16, kind="ExternalInput")
    wo = nc.dram_tensor("wo", [128, 4, D], BF16, kind="ExternalInput")
    ncs = nc.dram_tensor("ncs", [1, D], BF16, kind="ExternalInput")
    bvr = nc.dram_tensor("bvr", [1, D], BF16, kind="ExternalInput")
    b1p = nc.dram_tensor("b1p", [128, 4], F32, kind="ExternalInput")
    b2p = nc.dram_tensor("b2p", [128, 1], F32, kind="ExternalInput")
    kc = nc.dram_tensor("kc", [128, 1], F32, kind="ExternalInput")
    tblN = nc.dram_tensor("tblN", [128, 3, 4, 2, 32], F32, kind="ExternalInput")
    sgnN = nc.dram_tensor("sgnN", [128, 3, 4, 2, 32], BF16, kind="ExternalInput")
    tblF = nc.dram_tensor("tblF", [64, CH], F32, kind="ExternalInput")
    sgnF = nc.dram_tensor("sgnF", [64, CH], BF16, kind="ExternalInput")
    epsn = nc.dram_tensor("epsn", [128, 4], F32, kind="ExternalInput")
    res = nc.dram_tensor("res", [128, 4, D], F32, kind="ExternalInput")
    xn = nc.dram_tensor("xn", [128, 12, D], BF16, kind="ExternalInput")
    y = nc.dram_tensor("y", [CH, D], BF16, kind="ExternalOutput")

    with tile.TileContext(nc) as tc:
        with (
            tc.tile_pool(name="const", bufs=1) as cp_,
            tc.tile_pool(name="big", bufs=1) as bigp,
            tc.tile_pool(name="rot", bufs=3) as rot,
            tc.tile_pool(name="rot4", bufs=4) as rot4,
            tc.tile_pool(name="pmm", bufs=4, space="PSUM") as pmm,
            tc.tile_pool(name="pone", bufs=1, space="PSUM") as pone,
            tc.tile_pool(name="pintra", bufs=2, space="PSUM") as pintra,
            tc.tile_pool(name="ptr", bufs=1, space="PSUM") as ptrp,
        ):
            # ---- input loads, spread over queues so (a) starts ASAP ----
            xt_sb = bigp.tile([128, 4, T], BF16)
            w1_sb = cp_.tile([128, 4, D], BF16)
            w2_sb = cp_.tile([128, 4, K], BF16)
            wv_sb = cp_.tile([128, 4, D], BF16)
            wo_sb = cp_.tile([128, 4, D], BF16)
            res_sb = cp_.tile([128, 4, D], F32)
            bvr_sb = cp_.tile([1, D], BF16)
            ncs_sb = cp_.tile([1, D], BF16)
            b1p_sb = cp_.tile([128, 4], F32)
            b2p_sb = cp_.tile([128, 1], F32)
            kc_sb = cp_.tile([128, 1], F32)
            tblN_sb = cp_.tile([128, 3, 4, 2, 32], F32)
            sgnN_sb = cp_.tile([128, 3, 4, 2, 32], BF16)
            tblF_sb = cp_.tile([64, CH], F32)
            sgnF_sb = cp_.tile([64, CH], BF16)
            epsn_sb = cp_.tile([128, 4], F32)
            xn_sb = bigp.tile([128, 12, D], BF16)

            # dummy tanh first in the scalar program: its act-table load
            # runs right after the preamble, before the DMA issue backlog
            onesf = cp_.tile([128, 128], F32)
            nc.vector.memset(onesf[:], 1.0)
            dumb2 = cp_.tile([128, 128], BF16)
            nc.scalar.activation(
                dumb2[:], onesf[:], AF.Tanh, bias=onesf[:, 0:1], scale=1.0
            )

            # DMA plan (per-queue bandwidth ~83 GB/s; order = first use):
            # sync carries chunk-0 k01 + even chunks, gpsimd w1 + odd
            # chunks + late weights, scalar only small tensors.
            nc.sync.dma_start(xt_sb[:, 0, ts(0, CH)], xt[0][:, 0, :])
            nc.sync.dma_start(xt_sb[:, 1, ts(0, CH)], xt[0][:, 1, :])
            nc.sync.dma_start(xt_sb[:, 2, ts(0, CH)], xt[0][:, 2, :])
            nc.sync.dma_start(xt_sb[:, 0:2, ts(1, CH)], xt[1][:, 0:2, :])
            nc.sync.dma_start(xt_sb[:, 0:2, ts(2, CH)], xt[2][:, 0:2, :])
            nc.sync.dma_start(xt_sb[:, 0:2, ts(3, CH)], xt[3][:, 0:2, :])
            nc.sync.dma_start(xn_sb[:], xn[:])
            nc.sync.dma_start(ncs_sb[:], ncs[:])
            nc.sync.dma_start(bvr_sb[:], bvr[:])
            nc.sync.dma_start(epsn_sb[:], epsn[:])
            # gpsimd: w1 in k order, odd-chunk halves, then late weights
            nc.gpsimd.dma_start(w1_sb[:, 0:1, :], w1f[:])
            nc.gpsimd.dma_start(w1_sb[:, 1:2, :], w1[:, 1:2, :])
            nc.gpsimd.dma_start(xt_sb[:, 3, ts(0, CH)], xt[0][:, 3, :])
            nc.gpsimd.dma_start(xt_sb[:, 2:4, ts(1, CH)], xt[1][:, 2:4, :])
            nc.gpsimd.dma_start(xt_sb[:, 2:4, ts(2, CH)], xt[2][:, 2:4, :])
            nc.gpsimd.dma_start(xt_sb[:, 2:4, ts(3, CH)], xt[3][:, 2:4, :])
            nc.gpsimd.dma_start(wv_sb[:], wv[:])
            nc.gpsimd.dma_start(tblF_sb[:], tblF[:])
            nc.gpsimd.dma_start(sgnF_sb[:], sgnF[:])
            nc.gpsimd.dma_start(res_sb[:], res[:])
            nc.gpsimd.dma_start(wo_sb[:], wo[:])
            # scalar: tiny early tensors and the phase tables
            nc.scalar.dma_start(w2_sb[:], w2[:])
            nc.scalar.dma_start(w1_sb[:, 2:4, :], w1[:, 2:4, :])
            nc.scalar.dma_start(b1p_sb[:], b1p[:])
            nc.scalar.dma_start(kc_sb[:], kc[:])
            nc.scalar.dma_start(b2p_sb[:], b2p[:])
            nc.scalar.dma_start(tblN_sb[:], tblN[:])
            nc.scalar.dma_start(sgnN_sb[:], sgnN[:])

            onesf = cp_.tile([128, 128], F32)
            nc.vector.memset(onesf[:], 1.0)

            onesr = cp_.tile([1, 128], BF16)
            nc.vector.tensor_copy(onesr[:], onesf[0:1, :])
            onesc = cp_.tile([128, 1], BF16)
            nc.vector.tensor_copy(onesc[:], onesf[:, 0:1])

            identb = cp_.tile([128, 128], BF16)
            make_identity(nc, identb[:])

            # warm up the PE pstate while the first DMAs land: a dozen
            # throwaway matmuls ramp the clock so real matmuls start at speed
            wsrc = cp_.tile([128, CH], BF16)
            nc.vector.memset(wsrc[:], 0.0)
            pwarm = pmm.tile([128, CH], F32, tag="pmm", name="pwarm")
            for _ in range(8):
                nc.tensor.matmul(
                    pwarm[:], wsrc[:, 0:128], wsrc[:], start=True, stop=True
                )


            # triangular masks for intra-chunk causal attention (lhsT form:
            # tri[p, tr, y] = 1 iff y >= p + 128*tr)
            tri = cp_.tile([128, 4, CH], BF16)
            for tr in range(4):
                nc.gpsimd.memset(tri[:, tr, :], 0.0)
                nc.gpsimd.affine_select(
                    out=tri[:, tr, :], in_=tri[:, tr, :],
                    compare_op=ALU.is_gt, fill=1.0, base=128 * tr,
                    pattern=[[-1, CH]], channel_multiplier=1,
                )

            # ---- (a) h^T = tanh(W1^T x^T + b1) per chunk; (b) content tt,
            # deferred one chunk so the tanh latency hides under (a). The
            # natural-layout phases for each PREFIX chunk are produced as
            # soon as that chunk's content is ready (per-chunk transposes +
            # sin), so the prefix-state matmuls interleave with (a) and the
            # PE never waits on the phase chain. Chunk 3 (own chunk) only
            # needs the freq-major csc path. ----
            tt_sb = cp_.tile([128, CH], BF16)
            h_cks = [None] * 4
            value_sb = bigp.tile([128, 4, D], BF16)
            pvs = [None] * 4
            ttN2 = cp_.tile([128, 3, 4, 32], BF16)
            argN2 = cp_.tile([128, 3, 4, 2, 32], F32)
            sinN2 = cp_.tile([128, 3, 4, 2, 32], F32)
            # csm2N[p, c, b, path, f]: (path, f) contiguous so the pg lhsT
            # slice coalesces to a 2D [128, 64] access pattern
            csm2N = cp_.tile([128, 3, 4, 2, 32], BF16)
            pg = pone.tile([64, D], F32, tag="pst")

            def emit_b(c):
                pc = pintra.tile([32, CH], F32, tag="pintra", name=f"pc{c}")
                for k in range(4):
                    nc.tensor.matmul(
                        pc[:], w2_sb[:, k, :], h_cks[c][:, k, :],
                        start=(k == 0), stop=(k == 3),
                    )
                nc.scalar.activation(
                    tt_sb[32 * c : 32 * c + 32, :], pc[:], AF.Tanh,
                    bias=b2p_sb[0:32, :], scale=1.0,
                )

            def emit_natural_chain(c):
                # PE: 4 tiny transposes of this chunk's content rows; the
                # sins are batched after the last tanh (one table switch)
                ptc = ptrp.tile([128, 4, 32], BF16, tag="ptr", name=f"ptc{c}")
                for b in range(4):
                    nc.tensor.matmul(
                        ptc[:, b, :], tt_sb[32 * c : 32 * c + 32, ts(b, 128)],
                        identb[32 * c : 32 * c + 32, 32 * c : 32 * c + 32],
                        is_transpose=True,
                        skip_group_check=True,
                    )
                nc.vector.tensor_copy(ttN2[:, c], ptc[:])
                for path in range(2):
                    nc.vector.scalar_tensor_tensor(
                        out=argN2[:, c, :, path, :], in0=ttN2[:, c],
                        scalar=kc_val, in1=tblN_sb[:, c, :, path, :],
                        op0=ALU.mult, op1=ALU.add,
                    )

            def emit_pg(c):
                for bb in range(4):
                    nc.tensor.matmul(
                        pg[:], csm2N[:, c, bb, :, :], xn_sb[:, 4 * c + bb, :],
                        start=(c == 0 and bb == 0), stop=(c == 2 and bb == 3),
                    )

            def emit_value_mm(tt):
                pv = pmm.tile([128, D], F32, tag="pmm", name=f"pv{tt}")
                pvs[tt] = pv
                for k in range(4):
                    nc.tensor.matmul(
                        pv[:], xt_sb[:, k, ts(12 + tt, 128)], wv_sb[:, k, :],
                        start=(k == 0), stop=(zero_bv and k == 3),
                    )
                if not zero_bv:
                    nc.tensor.matmul(pv[:], onesr[:], bvr_sb[:], start=False, stop=True)

            def emit_value_copy(tt, eng):
                if eng == "s":
                    nc.scalar.copy(value_sb[:, tt, :], pvs[tt][:])
                else:
                    nc.vector.tensor_copy(value_sb[:, tt, :], pvs[tt][:])

            def emit_a(c):
                h_ck = rot.tile([128, 4, CH], BF16, tag="hck")
                h_cks[c] = h_ck
                if c == 0:
                    # two k-pair phases: consume the k-split chunk-0 DMAs as
                    # they land, and fire each tanh right after its k3
                    phs = [pmm.tile([128, CH], F32, tag="pmm", name=f"ph0_{d}")
                           for d in range(4)]
                    for k in range(2):
                        for dout in range(4):
                            nc.tensor.matmul(
                                phs[dout][:], w1_sb[:, k, ts(dout, 128)],
                                xt_sb[:, k, ts(0, CH)],
                                start=(k == 0), stop=False,
                            )
                    for dout in range(4):
                        nc.tensor.matmul(
                            phs[dout][:], w1_sb[:, 2, ts(dout, 128)],
                            xt_sb[:, 2, ts(0, CH)], start=False, stop=False,
                        )
                    for dout in range(4):
                        nc.tensor.matmul(
                            phs[dout][:], w1_sb[:, 3, ts(dout, 128)],
                            xt_sb[:, 3, ts(0, CH)], start=False, stop=True,
                        )
                        nc.scalar.activation(
                            h_ck[:, dout, :], phs[dout][:], AF.Tanh,
                            bias=b1p_sb[:, dout : dout + 1], scale=1.0,
                        )
                else:
                    for dout in range(4):
                        ph = pmm.tile([128, CH], F32, tag="pmm")
                        for k in range(4):
                            nc.tensor.matmul(
                                ph[:], w1_sb[:, k, ts(dout, 128)],
                                xt_sb[:, k, ts(c, CH)],
                                start=(k == 0), stop=(k == 3),
                            )
                        nc.scalar.activation(
                            h_ck[:, dout, :], ph[:], AF.Tanh,
                            bias=b1p_sb[:, dout : dout + 1], scale=1.0,
                        )

            emit_a(0)
            pf1 = pintra.tile([128, CH], F32, tag="pintra", name="pf1")
            for _ in range(4):
                nc.tensor.matmul(
                    pf1[:], wsrc[:, 0:128], wsrc[:], start=True, stop=True
                )
            emit_a(1)
            emit_a(2)
            emit_b(0)
            emit_natural_chain(0)
            emit_a(3)
            emit_b(1)
            emit_natural_chain(1)
            emit_b(2)
            emit_natural_chain(2)
            emit_b(3)

            # ---- freq-major phases for the own chunk (csc [64, CH]) ----
            ttF = cp_.tile([64, CH], BF16)
            argF = cp_.tile([64, CH], F32)
            sinF = cp_.tile([64, CH], F32)
            csc = cp_.tile([64, CH], BF16)
            nc.gpsimd.tensor_copy(ttF[0:32, :], tt_sb[96:128, :])
            nc.gpsimd.tensor_copy(ttF[32:64, :], tt_sb[96:128, :])
            nc.vector.scalar_tensor_tensor(
                out=argF[:], in0=ttF[:], scalar=kc_val, in1=tblF_sb[:],
                op0=ALU.mult, op1=ALU.add,
            )
            # one tanh->sin table switch, then every sin in a row. The
            # zero-valued bias token depends on ALL (b) tanh rows, which
            # pins the sins after the last tanh in the schedule.
            tok = cp_.tile([128, 1], F32)
            nc.vector.tensor_scalar_mul(tok[:], tt_sb[:, 0:1], 0.0)
            for c in range(3):
                nc.scalar.activation(
                    sinN2[:, c], argN2[:, c], AF.Sin, bias=tok[:]
                )
                nc.gpsimd.tensor_mul(csm2N[:, c], sinN2[:, c], sgnN_sb[:, c])
            nc.scalar.activation(sinF[:], argF[:], AF.Sin, bias=tok[0:64, :])
            nc.gpsimd.tensor_mul(csc[:], sinF[:], sgnF_sb[:])
            # preload the sqrt table after the last sin (gated by a token on
            # the final sin output); only Copy/Sqrt remain on scalar after
            # this, so the load hides under the retrieve matmuls
            tok2 = cp_.tile([128, 1], F32)
            nc.vector.tensor_scalar_mul(tok2[:], sinN2[:, 2, 0, 0, 0:1], 0.0)
            dsq = cp_.tile([128, 4], F32)
            nc.scalar.activation(dsq[:], onesf[:, 0:4], AF.Sqrt, bias=tok2[:])

            # PE: value fills while the sin batch runs, then pg + intra
            emit_value_mm(0)
            emit_value_mm(1)
            emit_value_mm(2)
            emit_value_mm(3)
            emit_value_copy(0, "v")
            pf2 = pintra.tile([128, CH], F32, tag="pintra", name="pf2")
            for _ in range(7):
                nc.tensor.matmul(
                    pf2[:], wsrc[:, 0:128], wsrc[:], start=True, stop=True
                )
            emit_pg(0)
            emit_pg(1)
            emit_pg(2)
            emit_value_copy(1, "s")

            # ---- (e) intra-chunk scores, triu-masked (own PSUM pool so the
            # value copies never gate them) ----
            p_sb = cp_.tile([128, 4, CH], BF16)
            for tr in range(4):
                psc = pintra.tile([128, CH], F32, tag="pintra")
                nc.tensor.matmul(
                    psc[:], csc[:, ts(tr, 128)], csc[:],
                    start=True, stop=True,
                )
                nc.vector.tensor_mul(p_sb[:, tr, :], psc[:], tri[:, tr, :])
            emit_value_copy(2, "v")
            emit_value_copy(3, "s")

            g_sb = cp_.tile([64, D], BF16)
            nc.vector.tensor_copy(g_sb[:], pg[:])
            gt_sb = cp_.tile([128, 4, 64], BF16)
            ptg = ptrp.tile([128, 4, 64], BF16, tag="ptr", name="ptg")
            for kk in range(4):
                nc.tensor.matmul(
                    ptg[:, kk, :], g_sb[:, ts(kk, 128)], identb[0:64, 0:64],
                    is_transpose=True, skip_group_check=True,
                )
            nc.vector.tensor_copy(gt_sb[:], ptg[:])
            pst = pone.tile([64, D], F32, tag="pst")
            for kk in range(4):
                nc.tensor.matmul(
                    pst[:], gt_sb[:, kk, :], wv_sb[:, kk, :],
                    start=(kk == 0), stop=(zero_bv and kk == 3),
                )
            if not zero_bv:
                # msum[j] = sum_l CS[l, j]; state += msum (x) bv
                pms = ptrp.tile([64, 1], F32, tag="ptr", name="pms")
                first = True
                for c in range(3):
                    for bb in range(4):
                        nc.tensor.matmul(
                            pms[:], csm2N[:, c, bb, :, :], onesc[:],
                            start=first, stop=(c == 2 and bb == 3),
                        )
                        first = False
                ms_sb = cp_.tile([64, 1], BF16)
                nc.vector.tensor_copy(ms_sb[:], pms[:])
                msT = cp_.tile([1, 64], BF16)
                ptm = ptrp.tile([128, 128], BF16, tag="ptr", name="ptm")
                nc.tensor.transpose(
                    ptm[0:1, 0:64], ms_sb[:], identb[0:64, 0:64]
                )
                nc.vector.tensor_copy(msT[:], ptm[0:1, 0:64])
                nc.tensor.matmul(pst[:], msT[:], bvr_sb[:], start=False, stop=True)
            state_sb = cp_.tile([64, D], BF16)
            nc.vector.tensor_copy(state_sb[:], pst[:])

            # ---- (f) retrieved^T [D, CH], stats interleaved ----
            retrT = cp_.tile([128, 4, CH], BF16)
            sq_sb = cp_.tile([128, 4, CH], BF16)
            ps_mean = pone.tile([1, CH], F32, tag="pst")
            ps_sq = pintra.tile([1, CH], F32, tag="pintra")

            def emit_retr(dd):
                pr = pmm.tile([128, CH], F32, tag="pmm")
                for tr in range(4):
                    nc.tensor.matmul(
                        pr[:], value_sb[:, tr, ts(dd, 128)], p_sb[:, tr, :],
                        start=(tr == 0), stop=False,
                    )
                nc.tensor.matmul(
                    pr[:], state_sb[:, ts(dd, 128)], csc[:],
                    start=False, stop=True,
                )
                if dd % 2 == 0:
                    nc.scalar.copy(retrT[:, dd, :], pr[:])
                else:
                    nc.vector.tensor_copy(retrT[:, dd, :], pr[:])
                nc.vector.tensor_mul(
                    sq_sb[:, dd, :], retrT[:, dd, :], retrT[:, dd, :]
                )

            def emit_stat(dd):
                nc.tensor.matmul(
                    ps_mean[0:1, :], onesc[:], retrT[:, dd, :],
                    start=(dd == 0), stop=(dd == 3),
                )
                nc.tensor.matmul(
                    ps_sq[0:1, :], onesc[:], sq_sb[:, dd, :],
                    start=(dd == 0), stop=(dd == 3),
                )

            emit_retr(0)
            emit_retr(1)
            emit_stat(0)
            emit_retr(2)
            emit_stat(1)
            emit_retr(3)
            emit_stat(2)
            emit_stat(3)

            # ---- LayerNorm rstd: bounce raw sums through DRAM into a
            # column layout [128, 8], then tiny per-partition math ----
            mu_n = cp_.tile([1, CH], BF16)
            nc.vector.tensor_scalar_mul(mu_n[:], ps_mean[0:1, :], 1.0 / D)
            stat_row = cp_.tile([1, 2 * CH], F32)
            nc.vector.tensor_copy(stat_row[0:1, 0:CH], ps_mean[0:1, :])
            nc.scalar.copy(stat_row[0:1, CH:], ps_sq[0:1, :])
            pstT = ptrp.tile([128, 8], F32, tag="ptr", name="pstT")
            for q in range(8):
                nc.tensor.matmul(
                    pstT[:, q : q + 1], stat_row[0:1, ts(q, 128)],
                    onesf[0:1, 0:1], is_transpose=True, skip_group_check=True,
                )
            statsT = cp_.tile([128, 8], F32)
            nc.vector.tensor_copy(statsT[:], pstT[:])
            muT = cp_.tile([128, 4], F32)
            nc.vector.tensor_scalar_mul(muT[:], statsT[:, 0:4], 1.0 / D)
            varT = cp_.tile([128, 4], F32)
            nc.vector.tensor_scalar_mul(varT[:], statsT[:, 4:8], 1.0 / D)
            mu2T = cp_.tile([128, 4], F32)
            nc.vector.tensor_mul(mu2T[:], muT[:], muT[:])
            nc.vector.tensor_sub(varT[:], varT[:], mu2T[:])
            nc.vector.tensor_add(varT[:], varT[:], epsn_sb[:])
            sdT = cp_.tile([128, 4], F32)
            nc.scalar.activation(sdT[:], varT[:], AF.Sqrt)
            rstdT = cp_.tile([128, 4], F32)
            nc.vector.reciprocal(rstdT[:], sdT[:])

            # ---- (h) out = rstd*(retr^T @ Wo' + mu*ncs) + res ----
            for tt in range(4):
                pho = pmm.tile([128, D], F32, tag="pmm")
                for ee in range(4):
                    nc.tensor.matmul(
                        pho[:], retrT[:, ee, ts(tt, 128)], wo_sb[:, ee, :],
                        start=(ee == 0), stop=False,
                    )
                nc.tensor.matmul(
                    pho[:], mu_n[0:1, ts(tt, 128)], ncs_sb[:],
                    start=False, stop=True,
                )
                out_t = rot4.tile([128, D], BF16, tag="outt")
                if tt % 2 == 0:
                    nc.vector.scalar_tensor_tensor(
                        out=out_t[:], in0=pho[:], scalar=rstdT[:, tt : tt + 1],
                        in1=res_sb[:, tt, :], op0=ALU.mult, op1=ALU.add,
                    )
                else:
                    tmp_t = rot4.tile([128, D], F32, tag="tmpt")
                    nc.scalar.mul(tmp_t[:], pho[:], rstdT[:, tt : tt + 1])
                    nc.vector.tensor_add(out_t[:], tmp_t[:], res_sb[:, tt, :])
                deng = nc.sync if tt % 2 == 0 else nc.scalar
                deng.dma_start(y[ts(tt, 128), :], out_t[:])

    nc.compile()
    return nc


def _get_nc(zero_bv: bool, kc_val: float):
    key = ("nc", zero_bv, round(kc_val, 9))
    if key not in _NC_CACHE:
        _NC_CACHE[key] = _build(zero_bv, kc_val)
    return _NC_CACHE[key]


def _prep_inputs(inputs):
    x = np.asarray(inputs["x"], np.float32)
    W1 = np.asarray(inputs["W1"], np.float32)
    b1 = np.asarray(inputs["b1"], np.float32)
    W2 = np.asarray(inputs["W2"], np.float32)
    b2 = np.asarray(inputs["b2"], np.float32)
    pos_scale = float(np.asarray(inputs["pos_scale"]).reshape(-1)[0])
    content_scale = float(np.asarray(inputs["content_scale"]).reshape(-1)[0])
    Wv = np.asarray(inputs["Wv"], np.float32)
    bv = np.asarray(inputs["bv"], np.float32)
    ln_g = np.asarray(inputs["ln_g"], np.float32)
    ln_b = np.asarray(inputs["ln_b"], np.float32)
    Wo = np.asarray(inputs["Wo"], np.float32)
    bo = np.asarray(inputs["bo"], np.float32)

    bf16 = ml_dtypes.bfloat16
    Wop = ln_g[:, None] * Wo                       # fold ln gain
    ncs_v = -Wop.sum(axis=0, dtype=np.float64).astype(np.float32)[None, :]
    res_base = (ln_b @ Wo + bo).astype(np.float32)  # fold ln bias + out bias

    # [p, k, out]: row Din = 128k+p  (exact SBUF layout, contiguous DMA)
    w1_t = np.ascontiguousarray(W1.reshape(4, 128, D).transpose(1, 0, 2))
    w2_t = np.ascontiguousarray(W2.reshape(4, 128, K).transpose(1, 0, 2))
    wv_t = np.ascontiguousarray(Wv.reshape(4, 128, D).transpose(1, 0, 2))
    wo_t = np.ascontiguousarray(Wop.reshape(4, 128, D).transpose(1, 0, 2))
    b1p = np.ascontiguousarray(b1.reshape(4, 128).T)
    b2p = np.tile(b2, 4)[:, None].astype(np.float32)
    kc = np.full((128, 1), np.pi * content_scale, np.float32)
    bvr = bv[None, :].astype(np.float32)

    freqs = 1.0 / (10000.0 ** (np.arange(K, dtype=np.float64) / K))

    def packN(a):
        # [T, K] -> [128p, 4c, 4b, 32f]: natural row l = 512c + 128b + p
        t = a.reshape(4, 4, 128, K)  # [c, b, p, f]
        return t.transpose(2, 0, 1, 3)

    in_maps = []
    for core in range(NCORE):
        b, i = divmod(core, 4)
        pad = 1536 - 512 * i
        nreal = 512 * (i + 1)
        xpad = np.zeros((T, D), np.float32)
        xpad[pad:] = x[b, :nreal]
        # xt dram layout: [c, 128, 4, CH]: [p, k] = Din 128k+p, per-chunk contiguous
        xt = np.ascontiguousarray(
            xpad.T.reshape(4, 128, 4, CH).transpose(2, 1, 0, 3))

        lidx = np.arange(T, dtype=np.float64) - pad
        ang = pos_scale * lidx[:, None] * freqs[None, :]      # [T, K]
        # S path: sin(ang + ct) -> fold ang = ps + pi*n, ps in [-pi/2, pi/2]
        n_s = np.round(ang / np.pi)
        ps_f = (ang - np.pi * n_s).astype(np.float32)
        sg_s = np.where(n_s % 2 == 0, 1.0, -1.0).astype(np.float32)
        # C path: cos(ang + ct) = sin(pi/2 + ang + ct)
        n_c = np.round((ang + np.pi / 2) / np.pi)
        pc_f = (ang + np.pi / 2 - np.pi * n_c).astype(np.float32)
        sg_c = np.where(n_c % 2 == 0, 1.0, -1.0).astype(np.float32)
        # padded rows contribute nothing: zero the signs (C = S = 0)
        sg_s[lidx < 0] = 0.0
        sg_c[lidx < 0] = 0.0
        ps_f[lidx < 0] = 0.0
        pc_f[lidx < 0] = 0.0

        # [128, 3, 4, 2, 32]: prefix chunks only, (path, f) innermost
        tblN_a = np.stack([packN(pc_f), packN(ps_f)], axis=3)[:, 0:3]
        sgnN_a = np.stack([packN(sg_c), packN(sg_s)], axis=3)[:, 0:3]
        # own chunk, freq-major [path*32+f, t]
        tblF_a = np.concatenate([pc_f[1536:].T, ps_f[1536:].T], axis=0)
        sgnF_a = np.concatenate([sg_c[1536:].T, sg_s[1536:].T], axis=0)
        epsn_r = (1e-5 * (np.arange(512 * i + 1, 512 * i + CH + 1,
                                    dtype=np.float64) * K)).astype(np.float32)
        epsn_a = np.ascontiguousarray(epsn_r.reshape(4, 128).T)  # [128p, 4tt]

        resc = (x[b, 512 * i : 512 * i + CH] + res_base[None, :]).astype(np.float32)

        xnat = np.ascontiguousarray(
            xpad[0:1536].reshape(12, 128, D).transpose(1, 0, 2))
        in_maps.append({
            "xt": xt.astype(bf16), "xn": xnat.astype(bf16),
            "w1f": np.ascontiguousarray(w1_t[:, 0:1, :].astype(bf16)),
            "w1": w1_t.astype(bf16), "w2": w2_t.astype(bf16),
            "wv": wv_t.astype(bf16), "wo": wo_t.astype(bf16),
            "ncs": ncs_v.astype(bf16), "bvr": bvr.astype(bf16),
            "b1p": b1p, "b2p": b2p, "kc": kc,
            "tblN": np.ascontiguousarray(tblN_a.astype(np.float32)),
            "sgnN": np.ascontiguousarray(sgnN_a.astype(bf16)),
            "tblF": np.ascontiguousarray(tblF_a.astype(np.float32)),
            "sgnF": np.ascontiguousarray(sgnF_a.astype(bf16)),
            "epsn": epsn_a,
            "res": np.ascontiguousarray(resc.reshape(4, 128, D).transpose(1, 0, 2)),
        })
    return in_maps


def kernel(**inputs) -> np.ndarray:
    global LAST_RESULT
    zero_bv = bool(np.all(np.asarray(inputs["bv"]) == 0.0))
    kc_val = float(np.pi * float(np.asarray(inputs["content_scale"]).reshape(-1)[0]))
    nc = _get_nc(zero_bv, kc_val)
    in_maps = _prep_inputs(inputs)
    result = run_bass_kernel_spmd(
        nc, in_maps, core_ids=list(range(NCORE)), **RUN_KWARGS
    )
    LAST_RESULT = result
    y = np.empty((B, L, D), np.float32)
    for core in range(NCORE):
        b, i = divmod(core, 4)
        y[b, 512 * i : 512 * i + CH] = np.asarray(
            result.results[core]["y"], dtype=np.float32)
    return y



# revision 10
# speedup vs baseline: 68.5983x; 68.5983x over previous
"""Trainium2 Bass kernel for nn_DualAddressingPhasor.

Math: the phasor cumsum-bind/retrieve is causal linear attention:
  retrieved[l] = sum_{l'<=l} (sum_k cos(phi_l,k - phi_l',k)) * value[l']
Per 512-row chunk this is (1) a carried state [2K, D] = CS^T @ value over
the prefix plus (2) intra-chunk attention triu(Cc@Cc^T + Sc@Sc^T) @ value_c.

Sharding: 8 cores = 2 batches x 4 sequence chunks of 512. Uniform SPMD
program; per-core variation is entirely in the data (right-aligned
zero-padded prefix, host-precomputed positional phase tables with zero
signs in the padding so padded rows contribute nothing).

Perf notes (vs the fp32r v1 at ~86us, now ~63-71us):
- all matmul operands bf16: fp32r matmuls self-load weights (~+70ns per
  512-row matmul); bf16 hits the 216ns peak with LDWEIGHTS overlapped.
- a dozen warmup matmuls on zeros ramp the PE clock out of its low
  pstate while the first DMAs land.
- per-chunk two-phase (k01/k23) accumulation in (a) consumes the
  k-split chunk-0 DMAs as they land; all x chunks are split across the
  sync/gpsimd queues (per-queue DMA bandwidth is only ~83 GB/s).
- the act-table RAM holds ONE function set at a time and the Tile
  scheduler freely interleaves ready work, so sins are token-gated
  behind the last tanh (one tanh->sin table switch instead of six) and
  the sqrt table is preloaded right after the sins, off-critical-path.
- natural-layout prefix phases come from 12 tiny per-chunk transposes
  of the content tile; chunk 3 (own chunk) only needs the freq-major
  path. LayerNorm stats are accumulated by matmul interleaved with the
  retrieve matmuls, transposed to column space via 8 tiny PE
  transposes, and the rstd math runs on [128,4] tiles while the output
  matmuls proceed.
"""
import sys

for _p in ("/opt/trn_rl_repo",):
    if _p not in sys.path:
        sys.path.append(_p)

import numpy as np
import ml_dtypes

import concourse.bacc as bacc
import concourse.tile as tile
import concourse.mybir as mybir
from concourse.bass import ts
from concourse.bass_utils import run_bass_kernel_spmd
from concourse.masks import make_identity

F32 = mybir.dt.float32
BF16 = mybir.dt.bfloat16
AF = mybir.ActivationFunctionType
ALU = mybir.AluOpType

D = 512
K = 32
B = 2
L = 2048
CH = 512          # chunk rows per core
T = 2048          # padded rows processed per core
NCORE = 8

_NC_CACHE = {}
LAST_RESULT = None
RUN_KWARGS = {}


def _build(zero_bv: bool, kc_val: float):
    nc = bacc.Bacc("TRN2", num_devices=NCORE)

    xt = nc.dram_tensor("xt", [4, 128, 4, CH], BF16, kind="ExternalInput")
    w1f = nc.dram_tensor("w1f", [128, 1, D], BF16, kind="ExternalInput")
    w1 = nc.dram_tensor("w1", [128, 4, D], BF16, kind="ExternalInput")
    w2 = nc.dram_tensor("w2", [128, 4, K], BF16, kind="ExternalInput")
    wv = nc.dram_tensor("wv", [128, 4, D], BF16, kind="ExternalInput")
    wo = nc.dram_tensor("wo", [128, 4, D], BF16, kind="ExternalInput")
    ncs = nc.dram_tensor("ncs", [1, D], BF16, kind="ExternalInput")
    bvr = nc.dram_tensor("bvr", [1, D], BF16, kind="ExternalInput")
    b1p = nc.dram_tensor("b1p", [128, 4], F32, kind="ExternalInput")
    b2p = nc.dram_tensor("b2p", [128, 1], F32, kind="ExternalInput")
    kc = nc.dram_tensor("kc", [128, 1], F32, kind="ExternalInput")
    tblN = nc.dram_tensor("tblN", [128, 3, 4, 2, 32], F32, kind="ExternalInput")
    sgnN = nc.dram_tensor("sgnN", [128, 3, 4, 2, 32], BF16, kind="ExternalInput")
    tblF = nc.dram_tensor("tblF", [64, CH], F32, kind="ExternalInput")
    sgnF = nc.dram_tensor("sgnF", [64, CH], BF16, kind="ExternalInput")
    epsn = nc.dram_tensor("epsn", [128, 4], F32, kind="ExternalInput")
    res = nc.dram_tensor("res", [128, 4, D], BF16, kind="ExternalInput")
    xn = nc.dram_tensor("xn", [128, 12, D], BF16, kind="ExternalInput")
    y = nc.dram_tensor("y", [CH, D], BF16, kind="ExternalOutput")

    with tile.TileContext(nc) as tc:
        with (
            tc.tile_pool(name="const", bufs=1) as cp_,
            tc.tile_pool(name="big", bufs=1) as bigp,
            tc.tile_pool(name="rot", bufs=3) as rot,
            tc.tile_pool(name="rot4", bufs=4) as rot4,
            tc.tile_pool(name="pmm", bufs=4, space="PSUM") as pmm,
            tc.tile_pool(name="pone", bufs=1, space="PSUM") as pone,
            tc.tile_pool(name="pintra", bufs=2, space="PSUM") as pintra,
            tc.tile_pool(name="ptr", bufs=1, space="PSUM") as ptrp,
        ):
            # ---- input loads, spread over queues so (a) starts ASAP ----
            xt_sb = bigp.tile([128, 4, T], BF16)
            w1_sb = cp_.tile([128, 4, D], BF16)
            w2_sb = cp_.tile([128, 4, K], BF16)
            wv_sb = cp_.tile([128, 4, D], BF16)
            wo_sb = cp_.tile([128, 4, D], BF16)
            res_sb = cp_.tile([128, 4, D], BF16)
            bvr_sb = cp_.tile([1, D], BF16)
            ncs_sb = cp_.tile([1, D], BF16)
            b1p_sb = cp_.tile([128, 4], F32)
            b2p_sb = cp_.tile([128, 1], F32)
            kc_sb = cp_.tile([128, 1], F32)
            tblN_sb = cp_.tile([128, 3, 4, 2, 32], F32)
            sgnN_sb = cp_.tile([128, 3, 4, 2, 32], BF16)
            tblF_sb = cp_.tile([64, CH], F32)
            sgnF_sb = cp_.tile([64, CH], BF16)
            epsn_sb = cp_.tile([128, 4], F32)
            xn_sb = bigp.tile([128, 12, D], BF16)

            # dummy tanh first in the scalar program: its act-table load
            # runs right after the preamble, before the DMA issue backlog
            onesf = cp_.tile([128, 128], F32)
            nc.vector.memset(onesf[:], 1.0)
            dumb2 = cp_.tile([128, 128], BF16)
            nc.scalar.activation(
                dumb2[:], onesf[:], AF.Tanh, bias=onesf[:, 0:1], scale=1.0
            )

            # DMA plan (per-queue bandwidth ~83 GB/s; order = first use):
            # sync carries chunk-0 k01 + even chunks, gpsimd w1 + odd
            # chunks + late weights, scalar only small tensors.
            nc.sync.dma_start(xt_sb[:, 0, ts(0, CH)], xt[0][:, 0, :])
            nc.sync.dma_start(xt_sb[:, 1, ts(0, CH)], xt[0][:, 1, :])
            nc.sync.dma_start(xt_sb[:, 2, ts(0, CH)], xt[0][:, 2, :])
            nc.sync.dma_start(xt_sb[:, 0:2, ts(1, CH)], xt[1][:, 0:2, :])
            nc.sync.dma_start(xt_sb[:, 0:2, ts(2, CH)], xt[2][:, 0:2, :])
            nc.sync.dma_start(xt_sb[:, 0:2, ts(3, CH)], xt[3][:, 0:2, :])
            nc.sync.dma_start(xn_sb[:], xn[:])
            nc.sync.dma_start(ncs_sb[:], ncs[:])
            nc.sync.dma_start(bvr_sb[:], bvr[:])
            nc.sync.dma_start(epsn_sb[:], epsn[:])
            # gpsimd: w1 in k order, odd-chunk halves, then late weights
            nc.gpsimd.dma_start(w1_sb[:, 0:1, :], w1f[:])
            nc.gpsimd.dma_start(w1_sb[:, 1:2, :], w1[:, 1:2, :])
            nc.gpsimd.dma_start(xt_sb[:, 3, ts(0, CH)], xt[0][:, 3, :])
            nc.gpsimd.dma_start(xt_sb[:, 2:4, ts(1, CH)], xt[1][:, 2:4, :])
            nc.gpsimd.dma_start(xt_sb[:, 2:4, ts(2, CH)], xt[2][:, 2:4, :])
            nc.gpsimd.dma_start(xt_sb[:, 2:4, ts(3, CH)], xt[3][:, 2:4, :])
            nc.gpsimd.dma_start(wv_sb[:], wv[:])
            nc.gpsimd.dma_start(tblF_sb[:], tblF[:])
            nc.gpsimd.dma_start(sgnF_sb[:], sgnF[:])
            nc.gpsimd.dma_start(wo_sb[:], wo[:])
            # scalar: tiny early tensors and the phase tables
            nc.scalar.dma_start(w2_sb[:], w2[:])
            nc.scalar.dma_start(w1_sb[:, 2:4, :], w1[:, 2:4, :])
            nc.scalar.dma_start(b1p_sb[:], b1p[:])
            nc.scalar.dma_start(kc_sb[:], kc[:])
            nc.scalar.dma_start(b2p_sb[:], b2p[:])
            nc.scalar.dma_start(tblN_sb[:], tblN[:])
            nc.scalar.dma_start(sgnN_sb[:], sgnN[:])
            nc.scalar.dma_start(res_sb[:], res[:])

            onesf = cp_.tile([128, 128], F32)
            nc.vector.memset(onesf[:], 1.0)

            onesr = cp_.tile([1, 128], BF16)
            nc.vector.tensor_copy(onesr[:], onesf[0:1, :])
            onesc = cp_.tile([128, 1], BF16)
            nc.vector.tensor_copy(onesc[:], onesf[:, 0:1])

            identb = cp_.tile([128, 128], BF16)
            make_identity(nc, identb[:])

            # warm up the PE pstate while the first DMAs land: a dozen
            # throwaway matmuls ramp the clock so real matmuls start at speed
            # (memset on gpsimd: vector is busy with const-pool init early)
            wsrc = cp_.tile([128, CH], BF16)
            nc.gpsimd.memset(wsrc[:], 0.0)
            pwarm = pmm.tile([128, CH], F32, tag="pmm", name="pwarm")
            for _ in range(8):
                nc.tensor.matmul(
                    pwarm[:], wsrc[:, 0:128], wsrc[:], start=True, stop=True
                )


            # triangular masks for intra-chunk causal attention (lhsT form:
            # tri[p, tr, y] = 1 iff y >= p + 128*tr)
            tri = cp_.tile([128, 4, CH], BF16)
            for tr in range(4):
                nc.gpsimd.memset(tri[:, tr, :], 0.0)
                nc.gpsimd.affine_select(
                    out=tri[:, tr, :], in_=tri[:, tr, :],
                    compare_op=ALU.is_gt, fill=1.0, base=128 * tr,
                    pattern=[[-1, CH]], channel_multiplier=1,
                )

            # ---- (a) h^T = tanh(W1^T x^T + b1) per chunk; (b) content tt,
            # deferred one chunk so the tanh latency hides under (a). The
            # natural-layout phases for each PREFIX chunk are produced as
            # soon as that chunk's content is ready (per-chunk transposes +
            # sin), so the prefix-state matmuls interleave with (a) and the
            # PE never waits on the phase chain. Chunk 3 (own chunk) only
            # needs the freq-major csc path. ----
            tt_sb = cp_.tile([128, CH], BF16)
            h_cks = [None] * 4
            value_sb = bigp.tile([128, 4, D], BF16)
            pvs = [None] * 4
            ttN2 = cp_.tile([128, 3, 4, 32], BF16)
            argN2 = cp_.tile([128, 3, 4, 2, 32], F32)
            sinN2 = cp_.tile([128, 3, 4, 2, 32], F32)
            # csm2N[p, c, b, path, f]: (path, f) contiguous so the pg lhsT
            # slice coalesces to a 2D [128, 64] access pattern
            csm2N = cp_.tile([128, 3, 4, 2, 32], BF16)
            pg = pone.tile([64, D], F32, tag="pst")

            def emit_b(c):
                pc = pintra.tile([32, CH], F32, tag="pintra", name=f"pc{c}")
                for k in range(4):
                    nc.tensor.matmul(
                        pc[:], w2_sb[:, k, :], h_cks[c][:, k, :],
                        start=(k == 0), stop=(k == 3),
                    )
                nc.scalar.activation(
                    tt_sb[32 * c : 32 * c + 32, :], pc[:], AF.Tanh,
                    bias=b2p_sb[0:32, :], scale=1.0,
                )

            def emit_natural_chain(c):
                # PE: 4 tiny transposes of this chunk's content rows; the
                # sins are batched after the last tanh (one table switch)
                ptc = ptrp.tile([128, 4, 32], BF16, tag="ptr", name=f"ptc{c}")
                for b in range(4):
                    nc.tensor.matmul(
                        ptc[:, b, :], tt_sb[32 * c : 32 * c + 32, ts(b, 128)],
                        identb[32 * c : 32 * c + 32, 32 * c : 32 * c + 32],
                        is_transpose=True,
                        skip_group_check=True,
                    )
                nc.vector.tensor_copy(ttN2[:, c], ptc[:])
                for path in range(2):
                    nc.vector.scalar_tensor_tensor(
                        out=argN2[:, c, :, path, :], in0=ttN2[:, c],
                        scalar=kc_val, in1=tblN_sb[:, c, :, path, :],
                        op0=ALU.mult, op1=ALU.add,
                    )

            def emit_pg(c):
                for bb in range(4):
                    nc.tensor.matmul(
                        pg[:], csm2N[:, c, bb, :, :], xn_sb[:, 4 * c + bb, :],
                        start=(c == 0 and bb == 0), stop=(c == 2 and bb == 3),
                    )

            def emit_value_mm(tt):
                pv = pmm.tile([128, D], F32, tag="pmm", name=f"pv{tt}")
                pvs[tt] = pv
                for k in range(4):
                    nc.tensor.matmul(
                        pv[:], xt_sb[:, k, ts(12 + tt, 128)], wv_sb[:, k, :],
                        start=(k == 0), stop=(zero_bv and k == 3),
                    )
                if not zero_bv:
                    nc.tensor.matmul(pv[:], onesr[:], bvr_sb[:], start=False, stop=True)

            def emit_value_copy(tt, eng):
                if eng == "s":
                    nc.scalar.copy(value_sb[:, tt, :], pvs[tt][:])
                else:
                    nc.vector.tensor_copy(value_sb[:, tt, :], pvs[tt][:])

            def emit_a(c):
                h_ck = rot.tile([128, 4, CH], BF16, tag="hck")
                h_cks[c] = h_ck
                if c == 0:
                    # two k-pair phases: consume the k-split chunk-0 DMAs as
                    # they land, and fire each tanh right after its k3
                    phs = [pmm.tile([128, CH], F32, tag="pmm", name=f"ph0_{d}")
                           for d in range(4)]
                    for k in range(2):
                        for dout in range(4):
                            nc.tensor.matmul(
                                phs[dout][:], w1_sb[:, k, ts(dout, 128)],
                                xt_sb[:, k, ts(0, CH)],
                                start=(k == 0), stop=False,
                            )
                    for dout in range(4):
                        nc.tensor.matmul(
                            phs[dout][:], w1_sb[:, 2, ts(dout, 128)],
                            xt_sb[:, 2, ts(0, CH)], start=False, stop=False,
                        )
                    for dout in range(4):
                        nc.tensor.matmul(
                            phs[dout][:], w1_sb[:, 3, ts(dout, 128)],
                            xt_sb[:, 3, ts(0, CH)], start=False, stop=True,
                        )
                        nc.scalar.activation(
                            h_ck[:, dout, :], phs[dout][:], AF.Tanh,
                            bias=b1p_sb[:, dout : dout + 1], scale=1.0,
                        )
                else:
                    for dout in range(4):
                        ph = pmm.tile([128, CH], F32, tag="pmm")
                        for k in range(4):
                            nc.tensor.matmul(
                                ph[:], w1_sb[:, k, ts(dout, 128)],
                                xt_sb[:, k, ts(c, CH)],
                                start=(k == 0), stop=(k == 3),
                            )
                        nc.scalar.activation(
                            h_ck[:, dout, :], ph[:], AF.Tanh,
                            bias=b1p_sb[:, dout : dout + 1], scale=1.0,
                        )

            emit_a(0)
            emit_a(1)
            emit_a(2)
            emit_b(0)
            emit_natural_chain(0)
            emit_a(3)
            emit_b(1)
            emit_natural_chain(1)
            emit_b(2)
            emit_natural_chain(2)
            emit_b(3)

            # ---- freq-major phases for the own chunk (csc [64, CH]) ----
            ttF = cp_.tile([64, CH], BF16)
            argF = cp_.tile([64, CH], F32)
            sinF = cp_.tile([64, CH], F32)
            csc = cp_.tile([64, CH], BF16)
            nc.gpsimd.tensor_copy(ttF[0:32, :], tt_sb[96:128, :])
            nc.gpsimd.tensor_copy(ttF[32:64, :], tt_sb[96:128, :])
            nc.vector.scalar_tensor_tensor(
                out=argF[:], in0=ttF[:], scalar=kc_val, in1=tblF_sb[:],
                op0=ALU.mult, op1=ALU.add,
            )
            # one tanh->sin table switch, then every sin in a row. The
            # zero-valued bias token depends on ALL (b) tanh rows, which
            # pins the sins after the last tanh in the schedule.
            tok = cp_.tile([128, 1], F32)
            nc.vector.tensor_scalar_mul(tok[:], tt_sb[:, 0:1], 0.0)
            for c in range(3):
                nc.scalar.activation(
                    sinN2[:, c], argN2[:, c], AF.Sin, bias=tok[:]
                )
                nc.gpsimd.tensor_mul(csm2N[:, c], sinN2[:, c], sgnN_sb[:, c])
            nc.scalar.activation(sinF[:], argF[:], AF.Sin, bias=tok[0:64, :])
            nc.gpsimd.tensor_mul(csc[:], sinF[:], sgnF_sb[:])
            # preload the sqrt table after the last sin (gated by a token on
            # the final sin output); only Copy/Sqrt remain on scalar after
            # this, so the load hides under the retrieve matmuls
            tok2 = cp_.tile([128, 1], F32)
            nc.vector.tensor_scalar_mul(tok2[:], sinN2[:, 2, 0, 0, 0:1], 0.0)
            dsq = cp_.tile([128, 4], F32)
            nc.scalar.activation(dsq[:], onesf[:, 0:4], AF.Sqrt, bias=tok2[:])

            # PE: value fills while the sin batch runs, then pg + intra
            emit_value_mm(0)
            emit_value_mm(1)
            emit_value_mm(2)
            emit_value_mm(3)
            emit_value_copy(0, "v")
            emit_pg(0)
            emit_pg(1)
            emit_pg(2)
            emit_value_copy(1, "s")

            # ---- (e) intra-chunk scores, triu-masked (own PSUM pool so the
            # value copies never gate them) ----
            p_sb = cp_.tile([128, 4, CH], BF16)
            for tr in range(4):
                psc = pintra.tile([128, CH], F32, tag="pintra")
                nc.tensor.matmul(
                    psc[:], csc[:, ts(tr, 128)], csc[:],
                    start=True, stop=True,
                )
                nc.vector.tensor_mul(p_sb[:, tr, :], psc[:], tri[:, tr, :])
            emit_value_copy(2, "v")
            emit_value_copy(3, "s")

            g_sb = cp_.tile([64, D], BF16)
            nc.vector.tensor_copy(g_sb[:], pg[:])
            gt_sb = cp_.tile([128, 4, 64], BF16)
            ptg = ptrp.tile([128, 4, 64], BF16, tag="ptr", name="ptg")
            for kk in range(4):
                nc.tensor.matmul(
                    ptg[:, kk, :], g_sb[:, ts(kk, 128)], identb[0:64, 0:64],
                    is_transpose=True, skip_group_check=True,
                )
            nc.vector.tensor_copy(gt_sb[:], ptg[:])
            pst = pone.tile([64, D], F32, tag="pst")
            for kk in range(4):
                nc.tensor.matmul(
                    pst[:], gt_sb[:, kk, :], wv_sb[:, kk, :],
                    start=(kk == 0), stop=(zero_bv and kk == 3),
                )
            if not zero_bv:
                # msum[j] = sum_l CS[l, j]; state += msum (x) bv
                pms = ptrp.tile([64, 1], F32, tag="ptr", name="pms")
                first = True
                for c in range(3):
                    for bb in range(4):
                        nc.tensor.matmul(
                            pms[:], csm2N[:, c, bb, :, :], onesc[:],
                            start=first, stop=(c == 2 and bb == 3),
                        )
                        first = False
                ms_sb = cp_.tile([64, 1], BF16)
                nc.vector.tensor_copy(ms_sb[:], pms[:])
                msT = cp_.tile([1, 64], BF16)
                ptm = ptrp.tile([128, 128], BF16, tag="ptr", name="ptm")
                nc.tensor.transpose(
                    ptm[0:1, 0:64], ms_sb[:], identb[0:64, 0:64]
                )
                nc.vector.tensor_copy(msT[:], ptm[0:1, 0:64])
                nc.tensor.matmul(pst[:], msT[:], bvr_sb[:], start=False, stop=True)
            state_sb = cp_.tile([64, D], BF16)
            nc.vector.tensor_copy(state_sb[:], pst[:])

            # ---- (f) retrieved^T [D, CH], stats interleaved ----
            retrT = cp_.tile([128, 4, CH], BF16)
            sq_sb = cp_.tile([128, 4, CH], BF16)
            ps_mean = pone.tile([1, CH], F32, tag="pst")
            ps_sq = pintra.tile([1, CH], F32, tag="pintra")

            def emit_retr(dd):
                pr = pmm.tile([128, CH], F32, tag="pmm")
                for tr in range(4):
                    nc.tensor.matmul(
                        pr[:], value_sb[:, tr, ts(dd, 128)], p_sb[:, tr, :],
                        start=(tr == 0), stop=False,
                    )
                nc.tensor.matmul(
                    pr[:], state_sb[:, ts(dd, 128)], csc[:],
                    start=False, stop=True,
                )
                if dd % 2 == 0:
                    nc.scalar.copy(retrT[:, dd, :], pr[:])
                else:
                    nc.vector.tensor_copy(retrT[:, dd, :], pr[:])
                nc.vector.tensor_mul(
                    sq_sb[:, dd, :], retrT[:, dd, :], retrT[:, dd, :]
                )

            def emit_stat(dd):
                nc.tensor.matmul(
                    ps_mean[0:1, :], onesc[:], retrT[:, dd, :],
                    start=(dd == 0), stop=(dd == 3),
                )
                nc.tensor.matmul(
                    ps_sq[0:1, :], onesc[:], sq_sb[:, dd, :],
                    start=(dd == 0), stop=(dd == 3),
                )

            emit_retr(0)
            emit_retr(1)
            emit_stat(0)
            emit_retr(2)
            emit_stat(1)
            emit_retr(3)
            emit_stat(2)
            emit_stat(3)

            # ---- LayerNorm rstd: bounce raw sums through DRAM into a
            # column layout [128, 8], then tiny per-partition math ----
            mu_n = cp_.tile([1, CH], BF16)
            nc.vector.tensor_scalar_mul(mu_n[:], ps_mean[0:1, :], 1.0 / D)
            stat_row = cp_.tile([1, 2 * CH], F32)
            nc.vector.tensor_copy(stat_row[0:1, 0:CH], ps_mean[0:1, :])
            nc.scalar.copy(stat_row[0:1, CH:], ps_sq[0:1, :])
            pstT = ptrp.tile([128, 8], F32, tag="ptr", name="pstT")
            for q in range(8):
                nc.tensor.matmul(
                    pstT[:, q : q + 1], stat_row[0:1, ts(q, 128)],
                    onesf[0:1, 0:1], is_transpose=True, skip_group_check=True,
                )
            statsT = cp_.tile([128, 8], F32)
            nc.vector.tensor_copy(statsT[:], pstT[:])
            muT = cp_.tile([128, 4], F32)
            nc.vector.tensor_scalar_mul(muT[:], statsT[:, 0:4], 1.0 / D)
            varT = cp_.tile([128, 4], F32)
            nc.vector.tensor_scalar_mul(varT[:], statsT[:, 4:8], 1.0 / D)
            mu2T = cp_.tile([128, 4], F32)
            nc.vector.tensor_mul(mu2T[:], muT[:], muT[:])
            nc.vector.tensor_sub(varT[:], varT[:], mu2T[:])
            nc.vector.tensor_add(varT[:], varT[:], epsn_sb[:])
            sdT = cp_.tile([128, 4], F32)
            nc.scalar.activation(sdT[:], varT[:], AF.Sqrt)
            rstdT = cp_.tile([128, 4], F32)
            nc.vector.reciprocal(rstdT[:], sdT[:])

            # ---- (h) out = rstd*(retr^T @ Wo' + mu*ncs) + res ----
            for tt in range(4):
                pho = pmm.tile([128, D], F32, tag="pmm")
                for ee in range(4):
                    nc.tensor.matmul(
                        pho[:], retrT[:, ee, ts(tt, 128)], wo_sb[:, ee, :],
                        start=(ee == 0), stop=False,
                    )
                nc.tensor.matmul(
                    pho[:], mu_n[0:1, ts(tt, 128)], ncs_sb[:],
                    start=False, stop=True,
                )
                out_t = rot4.tile([128, D], BF16, tag="outt")
                if tt % 2 == 0:
                    nc.vector.scalar_tensor_tensor(
                        out=out_t[:], in0=pho[:], scalar=rstdT[:, tt : tt + 1],
                        in1=res_sb[:, tt, :], op0=ALU.mult, op1=ALU.add,
                    )
                else:
                    tmp_t = rot4.tile([128, D], F32, tag="tmpt")
                    nc.scalar.mul(tmp_t[:], pho[:], rstdT[:, tt : tt + 1])
                    nc.vector.tensor_add(out_t[:], tmp_t[:], res_sb[:, tt, :])
                nc.sync.dma_start(y[ts(tt, 128), :], out_t[:])

    nc.compile()
    return nc


def _get_nc(zero_bv: bool, kc_val: float):
    key = ("nc", zero_bv, round(kc_val, 9))
    if key not in _NC_CACHE:
        _NC_CACHE[key] = _build(zero_bv, kc_val)
    return _NC_CACHE[key]


def _prep_inputs(inputs):
    x = np.asarray(inputs["x"], np.float32)
    W1 = np.asarray(inputs["W1"], np.float32)
    b1 = np.asarray(inputs["b1"], np.float32)
    W2 = np.asarray(inputs["W2"], np.float32)
    b2 = np.asarray(inputs["b2"], np.float32)
    pos_scale = float(np.asarray(inputs["pos_scale"]).reshape(-1)[0])
    content_scale = float(np.asarray(inputs["content_scale"]).reshape(-1)[0])
    Wv = np.asarray(inputs["Wv"], np.float32)
    bv = np.asarray(inputs["bv"], np.float32)
    ln_g = np.asarray(inputs["ln_g"], np.float32)
    ln_b = np.asarray(inputs["ln_b"], np.float32)
    Wo = np.asarray(inputs["Wo"], np.float32)
    bo = np.asarray(inputs["bo"], np.float32)

    bf16 = ml_dtypes.bfloat16
    Wop = ln_g[:, None] * Wo                       # fold ln gain
    ncs_v = -Wop.sum(axis=0, dtype=np.float64).astype(np.float32)[None, :]
    res_base = (ln_b @ Wo + bo).astype(np.float32)  # fold ln bias + out bias

    # [p, k, out]: row Din = 128k+p  (exact SBUF layout, contiguous DMA)
    w1_t = np.ascontiguousarray(W1.reshape(4, 128, D).transpose(1, 0, 2))
    w2_t = np.ascontiguousarray(W2.reshape(4, 128, K).transpose(1, 0, 2))
    wv_t = np.ascontiguousarray(Wv.reshape(4, 128, D).transpose(1, 0, 2))
    wo_t = np.ascontiguousarray(Wop.reshape(4, 128, D).transpose(1, 0, 2))
    b1p = np.ascontiguousarray(b1.reshape(4, 128).T)
    b2p = np.tile(b2, 4)[:, None].astype(np.float32)
    kc = np.full((128, 1), np.pi * content_scale, np.float32)
    bvr = bv[None, :].astype(np.float32)

    freqs = 1.0 / (10000.0 ** (np.arange(K, dtype=np.float64) / K))

    def packN(a):
        # [T, K] -> [128p, 4c, 4b, 32f]: natural row l = 512c + 128b + p
        t = a.reshape(4, 4, 128, K)  # [c, b, p, f]
        return t.transpose(2, 0, 1, 3)

    in_maps = []
    for core in range(NCORE):
        b, i = divmod(core, 4)
        pad = 1536 - 512 * i
        nreal = 512 * (i + 1)
        xpad = np.zeros((T, D), np.float32)
        xpad[pad:] = x[b, :nreal]
        # xt dram layout: [c, 128, 4, CH]: [p, k] = Din 128k+p, per-chunk contiguous
        xt = np.ascontiguousarray(
            xpad.T.reshape(4, 128, 4, CH).transpose(2, 1, 0, 3))

        lidx = np.arange(T, dtype=np.float64) - pad
        ang = pos_scale * lidx[:, None] * freqs[None, :]      # [T, K]
        # S path: sin(ang + ct) -> fold ang = ps + pi*n, ps in [-pi/2, pi/2]
        n_s = np.round(ang / np.pi)
        ps_f = (ang - np.pi * n_s).astype(np.float32)
        sg_s = np.where(n_s % 2 == 0, 1.0, -1.0).astype(np.float32)
        # C path: cos(ang + ct) = sin(pi/2 + ang + ct)
        n_c = np.round((ang + np.pi / 2) / np.pi)
        pc_f = (ang + np.pi / 2 - np.pi * n_c).astype(np.float32)
        sg_c = np.where(n_c % 2 == 0, 1.0, -1.0).astype(np.float32)
        # padded rows contribute nothing: zero the signs (C = S = 0)
        sg_s[lidx < 0] = 0.0
        sg_c[lidx < 0] = 0.0
        ps_f[lidx < 0] = 0.0
        pc_f[lidx < 0] = 0.0

        # [128, 3, 4, 2, 32]: prefix chunks only, (path, f) innermost
        tblN_a = np.stack([packN(pc_f), packN(ps_f)], axis=3)[:, 0:3]
        sgnN_a = np.stack([packN(sg_c), packN(sg_s)], axis=3)[:, 0:3]
        # own chunk, freq-major [path*32+f, t]
        tblF_a = np.concatenate([pc_f[1536:].T, ps_f[1536:].T], axis=0)
        sgnF_a = np.concatenate([sg_c[1536:].T, sg_s[1536:].T], axis=0)
        epsn_r = (1e-5 * (np.arange(512 * i + 1, 512 * i + CH + 1,
                                    dtype=np.float64) * K)).astype(np.float32)
        epsn_a = np.ascontiguousarray(epsn_r.reshape(4, 128).T)  # [128p, 4tt]

        resc = (x[b, 512 * i : 512 * i + CH] + res_base[None, :]).astype(np.float32)

        xnat = np.ascontiguousarray(
            xpad[0:1536].reshape(12, 128, D).transpose(1, 0, 2))
        in_maps.append({
            "xt": xt.astype(bf16), "xn": xnat.astype(bf16),
            "w1f": np.ascontiguousarray(w1_t[:, 0:1, :].astype(bf16)),
            "w1": w1_t.astype(bf16), "w2": w2_t.astype(bf16),
            "wv": wv_t.astype(bf16), "wo": wo_t.astype(bf16),
            "ncs": ncs_v.astype(bf16), "bvr": bvr.astype(bf16),
            "b1p": b1p, "b2p": b2p, "kc": kc,
            "tblN": np.ascontiguousarray(tblN_a.astype(np.float32)),
            "sgnN": np.ascontiguousarray(sgnN_a.astype(bf16)),
            "tblF": np.ascontiguousarray(tblF_a.astype(np.float32)),
            "sgnF": np.ascontiguousarray(sgnF_a.astype(bf16)),
            "epsn": epsn_a,
            "res": np.ascontiguousarray(
                resc.reshape(4, 128, D).transpose(1, 0, 2)).astype(bf16),
        })
    return in_maps


def kernel(**inputs) -> np.ndarray:
    global LAST_RESULT
    zero_bv = bool(np.all(np.asarray(inputs["bv"]) == 0.0))
    kc_val = float(np.pi * float(np.asarray(inputs["content_scale"]).reshape(-1)[0]))
    nc = _get_nc(zero_bv, kc_val)
    in_maps = _prep_inputs(inputs)
    result = run_bass_kernel_spmd(
        nc, in_maps, core_ids=list(range(NCORE)), **RUN_KWARGS
    )
    LAST_RESULT = result
    y = np.empty((B, L, D), np.float32)
    for core in range(NCORE):
        b, i = divmod(core, 4)
        y[b, 512 * i : 512 * i + CH] = np.asarray(
            result.results[core]["y"], dtype=np.float32)
    return y



# revision 12
# speedup vs baseline: 70.8615x; 1.0330x over previous
"""Trainium2 Bass kernel for nn_DualAddressingPhasor.

Math: the phasor cumsum-bind/retrieve is causal linear attention:
  retrieved[l] = sum_{l'<=l} (sum_k cos(phi_l,k - phi_l',k)) * value[l']
Per 512-row chunk this is (1) a carried state [2K, D] = CS^T @ value over
the prefix plus (2) intra-chunk attention triu(Cc@Cc^T + Sc@Sc^T) @ value_c.

Sharding: 8 cores = 2 batches x 4 sequence chunks of 512. Uniform SPMD
program; per-core variation is entirely in the data (right-aligned
zero-padded prefix, host-precomputed positional phase tables with zero
signs in the padding so padded rows contribute nothing).

Perf notes (vs the fp32r v1 at ~86us, now ~63-71us):
- all matmul operands bf16: fp32r matmuls self-load weights (~+70ns per
  512-row matmul); bf16 hits the 216ns peak with LDWEIGHTS overlapped.
- a dozen warmup matmuls on zeros ramp the PE clock out of its low
  pstate while the first DMAs land.
- per-chunk two-phase (k01/k23) accumulation in (a) consumes the
  k-split chunk-0 DMAs as they land; all x chunks are split across the
  sync/gpsimd queues (per-queue DMA bandwidth is only ~83 GB/s).
- the act-table RAM holds ONE function set at a time and the Tile
  scheduler freely interleaves ready work, so sins are token-gated
  behind the last tanh (one tanh->sin table switch instead of six) and
  the sqrt table is preloaded right after the sins, off-critical-path.
- natural-layout prefix phases come from 12 tiny per-chunk transposes
  of the content tile; chunk 3 (own chunk) only needs the freq-major
  path. LayerNorm stats are accumulated by matmul interleaved with the
  retrieve matmuls, transposed to column space via 8 tiny PE
  transposes, and the rstd math runs on [128,4] tiles while the output
  matmuls proceed.
"""
import sys

for _p in ("/opt/trn_rl_repo",):
    if _p not in sys.path:
        sys.path.append(_p)

import numpy as np
import ml_dtypes

import concourse.bacc as bacc
import concourse.tile as tile
import concourse.mybir as mybir
from concourse.bass import ts
from concourse.bass_utils import run_bass_kernel_spmd
from concourse.masks import make_identity

F32 = mybir.dt.float32
BF16 = mybir.dt.bfloat16
AF = mybir.ActivationFunctionType
ALU = mybir.AluOpType

D = 512
K = 32
B = 2
L = 2048
CH = 512          # chunk rows per core
T = 2048          # padded rows processed per core
NCORE = 8

_NC_CACHE = {}
LAST_RESULT = None
RUN_KWARGS = {}


def _build(zero_bv: bool, kc_val: float):
    nc = bacc.Bacc("TRN2", num_devices=NCORE)

    xt = nc.dram_tensor("xt", [4, 128, 4, CH], BF16, kind="ExternalInput")
    w1f = nc.dram_tensor("w1f", [128, 1, D], BF16, kind="ExternalInput")
    w1 = nc.dram_tensor("w1", [128, 4, D], BF16, kind="ExternalInput")
    w2 = nc.dram_tensor("w2", [128, 4, K], BF16, kind="ExternalInput")
    wv = nc.dram_tensor("wv", [128, 4, D], BF16, kind="ExternalInput")
    wo = nc.dram_tensor("wo", [128, 4, D], BF16, kind="ExternalInput")
    ncs = nc.dram_tensor("ncs", [1, D], BF16, kind="ExternalInput")
    bvr = nc.dram_tensor("bvr", [1, D], BF16, kind="ExternalInput")
    b1p = nc.dram_tensor("b1p", [128, 4], F32, kind="ExternalInput")
    b2p = nc.dram_tensor("b2p", [128, 1], F32, kind="ExternalInput")
    kc = nc.dram_tensor("kc", [128, 1], F32, kind="ExternalInput")
    tblN = nc.dram_tensor("tblN", [128, 3, 4, 2, 32], F32, kind="ExternalInput")
    sgnN = nc.dram_tensor("sgnN", [128, 3, 4, 2, 32], BF16, kind="ExternalInput")
    tblF = nc.dram_tensor("tblF", [64, CH], F32, kind="ExternalInput")
    sgnF = nc.dram_tensor("sgnF", [64, CH], BF16, kind="ExternalInput")
    epsn = nc.dram_tensor("epsn", [128, 4], F32, kind="ExternalInput")
    res = nc.dram_tensor("res", [128, 4, D], BF16, kind="ExternalInput")
    xn = nc.dram_tensor("xn", [128, 12, D], BF16, kind="ExternalInput")
    y = nc.dram_tensor("y", [CH, D], BF16, kind="ExternalOutput")

    with tile.TileContext(nc) as tc:
        with (
            tc.tile_pool(name="const", bufs=1) as cp_,
            tc.tile_pool(name="big", bufs=1) as bigp,
            tc.tile_pool(name="rot", bufs=3) as rot,
            tc.tile_pool(name="rot4", bufs=4) as rot4,
            tc.tile_pool(name="pmm", bufs=4, space="PSUM") as pmm,
            tc.tile_pool(name="pone", bufs=1, space="PSUM") as pone,
            tc.tile_pool(name="pintra", bufs=2, space="PSUM") as pintra,
            tc.tile_pool(name="ptr", bufs=1, space="PSUM") as ptrp,
        ):
            # ---- input loads, spread over queues so (a) starts ASAP ----
            xt_sb = bigp.tile([128, 4, T], BF16)
            w1_sb = cp_.tile([128, 4, D], BF16)
            w2_sb = cp_.tile([128, 4, K], BF16)
            wv_sb = cp_.tile([128, 4, D], BF16)
            wo_sb = cp_.tile([128, 4, D], BF16)
            res_sb = cp_.tile([128, 4, D], BF16)
            bvr_sb = cp_.tile([1, D], BF16)
            ncs_sb = cp_.tile([1, D], BF16)
            b1p_sb = cp_.tile([128, 4], F32)
            b2p_sb = cp_.tile([128, 1], F32)
            kc_sb = cp_.tile([128, 1], F32)
            tblN_sb = cp_.tile([128, 3, 4, 2, 32], F32)
            sgnN_sb = cp_.tile([128, 3, 4, 2, 32], BF16)
            tblF_sb = cp_.tile([64, CH], F32)
            sgnF_sb = cp_.tile([64, CH], BF16)
            epsn_sb = cp_.tile([128, 4], F32)
            xn_sb = bigp.tile([128, 12, D], BF16)

            # dummy tanh first in the scalar program: its act-table load
            # runs right after the preamble, before the DMA issue backlog
            wsrc = cp_.tile([128, CH], BF16)
            nc.vector.memset(wsrc[:], 0.0)
            onesf = cp_.tile([128, 128], F32)
            nc.vector.memset(onesf[:], 1.0)
            dumb2 = cp_.tile([128, 128], BF16)
            nc.scalar.activation(
                dumb2[:], onesf[:], AF.Tanh, bias=onesf[:, 0:1], scale=1.0
            )

            # DMA plan (per-queue bandwidth ~83 GB/s; order = first use):
            # sync carries chunk-0 k01 + even chunks, gpsimd w1 + odd
            # chunks + late weights, scalar only small tensors.
            nc.sync.dma_start(xt_sb[:, 0, ts(0, CH)], xt[0][:, 0, :])
            nc.sync.dma_start(xt_sb[:, 1, ts(0, CH)], xt[0][:, 1, :])
            nc.sync.dma_start(xt_sb[:, 2, ts(0, CH)], xt[0][:, 2, :])
            nc.sync.dma_start(xt_sb[:, 0:2, ts(1, CH)], xt[1][:, 0:2, :])
            nc.sync.dma_start(xt_sb[:, 0:2, ts(2, CH)], xt[2][:, 0:2, :])
            nc.sync.dma_start(xt_sb[:, 0:2, ts(3, CH)], xt[3][:, 0:2, :])
            nc.sync.dma_start(xn_sb[:], xn[:])
            nc.sync.dma_start(ncs_sb[:], ncs[:])
            nc.sync.dma_start(bvr_sb[:], bvr[:])
            nc.sync.dma_start(epsn_sb[:], epsn[:])
            # gpsimd: w1 in k order, odd-chunk halves, then late weights
            nc.gpsimd.dma_start(w1_sb[:, 0:1, :], w1f[:])
            nc.gpsimd.dma_start(w1_sb[:, 1:2, :], w1[:, 1:2, :])
            nc.gpsimd.dma_start(xt_sb[:, 3, ts(0, CH)], xt[0][:, 3, :])
            nc.gpsimd.dma_start(xt_sb[:, 2:4, ts(1, CH)], xt[1][:, 2:4, :])
            nc.gpsimd.dma_start(xt_sb[:, 2:4, ts(2, CH)], xt[2][:, 2:4, :])
            nc.gpsimd.dma_start(xt_sb[:, 2:4, ts(3, CH)], xt[3][:, 2:4, :])
            nc.gpsimd.dma_start(wv_sb[:], wv[:])
            nc.gpsimd.dma_start(tblF_sb[:], tblF[:])
            nc.gpsimd.dma_start(sgnF_sb[:], sgnF[:])
            nc.gpsimd.dma_start(wo_sb[:], wo[:])
            # scalar: tiny early tensors and the phase tables
            nc.scalar.dma_start(w2_sb[:], w2[:])
            nc.scalar.dma_start(w1_sb[:, 2:4, :], w1[:, 2:4, :])
            nc.scalar.dma_start(b1p_sb[:], b1p[:])
            nc.scalar.dma_start(kc_sb[:], kc[:])
            nc.scalar.dma_start(b2p_sb[:], b2p[:])
            nc.scalar.dma_start(tblN_sb[:], tblN[:])
            nc.scalar.dma_start(sgnN_sb[:], sgnN[:])
            nc.scalar.dma_start(res_sb[:], res[:])

            onesf = cp_.tile([128, 128], F32)
            nc.vector.memset(onesf[:], 1.0)

            onesr = cp_.tile([1, 128], BF16)
            nc.vector.tensor_copy(onesr[:], onesf[0:1, :])
            onesc = cp_.tile([128, 1], BF16)
            nc.vector.tensor_copy(onesc[:], onesf[:, 0:1])

            identb = cp_.tile([128, 128], BF16)
            make_identity(nc, identb[:])

            # warm up the PE pstate while the first DMAs land: a dozen
            # throwaway matmuls ramp the clock so real matmuls start at speed
            pwarm = pmm.tile([128, CH], F32, tag="pmm", name="pwarm")
            for _ in range(8):
                nc.tensor.matmul(
                    pwarm[:], wsrc[:, 0:128], wsrc[:], start=True, stop=True
                )


            # triangular masks for intra-chunk causal attention (lhsT form:
            # tri[p, tr, y] = 1 iff y >= p + 128*tr)
            tri = cp_.tile([128, 4, CH], BF16)
            for tr in range(4):
                nc.gpsimd.memset(tri[:, tr, :], 0.0)
                nc.gpsimd.affine_select(
                    out=tri[:, tr, :], in_=tri[:, tr, :],
                    compare_op=ALU.is_gt, fill=1.0, base=128 * tr,
                    pattern=[[-1, CH]], channel_multiplier=1,
                )

            # ---- (a) h^T = tanh(W1^T x^T + b1) per chunk; (b) content tt,
            # deferred one chunk so the tanh latency hides under (a). The
            # natural-layout phases for each PREFIX chunk are produced as
            # soon as that chunk's content is ready (per-chunk transposes +
            # sin), so the prefix-state matmuls interleave with (a) and the
            # PE never waits on the phase chain. Chunk 3 (own chunk) only
            # needs the freq-major csc path. ----
            tt_sb = cp_.tile([128, CH], BF16)
            h_cks = [None] * 4
            value_sb = bigp.tile([128, 4, D], BF16)
            pvs = [None] * 4
            ttN2 = cp_.tile([128, 3, 4, 32], BF16)
            argN2 = cp_.tile([128, 3, 4, 2, 32], F32)
            sinN2 = cp_.tile([128, 3, 4, 2, 32], F32)
            # csm2N[p, c, b, path, f]: (path, f) contiguous so the pg lhsT
            # slice coalesces to a 2D [128, 64] access pattern
            csm2N = cp_.tile([128, 3, 4, 2, 32], BF16)
            pg = pone.tile([64, D], F32, tag="pst")

            def emit_b(c):
                pc = pintra.tile([32, CH], F32, tag="pintra", name=f"pc{c}")
                for k in range(4):
                    nc.tensor.matmul(
                        pc[:], w2_sb[:, k, :], h_cks[c][:, k, :],
                        start=(k == 0), stop=(k == 3),
                    )
                nc.scalar.activation(
                    tt_sb[32 * c : 32 * c + 32, :], pc[:], AF.Tanh,
                    bias=b2p_sb[0:32, :], scale=1.0,
                )

            def emit_natural_chain(c):
                # PE: 4 tiny transposes of this chunk's content rows; the
                # sins are batched after the last tanh (one table switch)
                ptc = ptrp.tile([128, 4, 32], BF16, tag="ptr", name=f"ptc{c}")
                for b in range(4):
                    nc.tensor.matmul(
                        ptc[:, b, :], tt_sb[32 * c : 32 * c + 32, ts(b, 128)],
                        identb[32 * c : 32 * c + 32, 32 * c : 32 * c + 32],
                        is_transpose=True,
                        skip_group_check=True,
                    )
                nc.vector.tensor_copy(ttN2[:, c], ptc[:])
                for path in range(2):
                    nc.vector.scalar_tensor_tensor(
                        out=argN2[:, c, :, path, :], in0=ttN2[:, c],
                        scalar=kc_val, in1=tblN_sb[:, c, :, path, :],
                        op0=ALU.mult, op1=ALU.add,
                    )

            def emit_pg(c):
                for bb in range(4):
                    nc.tensor.matmul(
                        pg[:], csm2N[:, c, bb, :, :], xn_sb[:, 4 * c + bb, :],
                        start=(c == 0 and bb == 0), stop=(c == 2 and bb == 3),
                    )

            def emit_value_mm(tt):
                pv = pmm.tile([128, D], F32, tag="pmm", name=f"pv{tt}")
                pvs[tt] = pv
                for k in range(4):
                    nc.tensor.matmul(
                        pv[:], xt_sb[:, k, ts(12 + tt, 128)], wv_sb[:, k, :],
                        start=(k == 0), stop=(zero_bv and k == 3),
                    )
                if not zero_bv:
                    nc.tensor.matmul(pv[:], onesr[:], bvr_sb[:], start=False, stop=True)

            def emit_value_copy(tt, eng):
                if eng == "s":
                    nc.scalar.copy(value_sb[:, tt, :], pvs[tt][:])
                else:
                    nc.vector.tensor_copy(value_sb[:, tt, :], pvs[tt][:])

            def emit_a(c):
                h_ck = rot.tile([128, 4, CH], BF16, tag="hck")
                h_cks[c] = h_ck
                if c == 0:
                    # two k-pair phases: consume the k-split chunk-0 DMAs as
                    # they land, and fire each tanh right after its k3
                    phs = [pmm.tile([128, CH], F32, tag="pmm", name=f"ph0_{d}")
                           for d in range(4)]
                    for k in range(2):
                        for dout in range(4):
                            nc.tensor.matmul(
                                phs[dout][:], w1_sb[:, k, ts(dout, 128)],
                                xt_sb[:, k, ts(0, CH)],
                                start=(k == 0), stop=False,
                            )
                    for dout in range(4):
                        nc.tensor.matmul(
                            phs[dout][:], w1_sb[:, 2, ts(dout, 128)],
                            xt_sb[:, 2, ts(0, CH)], start=False, stop=False,
                        )
                    for dout in range(4):
                        nc.tensor.matmul(
                            phs[dout][:], w1_sb[:, 3, ts(dout, 128)],
                            xt_sb[:, 3, ts(0, CH)], start=False, stop=True,
                        )
                        nc.scalar.activation(
                            h_ck[:, dout, :], phs[dout][:], AF.Tanh,
                            bias=b1p_sb[:, dout : dout + 1], scale=1.0,
                        )
                else:
                    for dout in range(4):
                        ph = pmm.tile([128, CH], F32, tag="pmm")
                        for k in range(4):
                            nc.tensor.matmul(
                                ph[:], w1_sb[:, k, ts(dout, 128)],
                                xt_sb[:, k, ts(c, CH)],
                                start=(k == 0), stop=(k == 3),
                            )
                        nc.scalar.activation(
                            h_ck[:, dout, :], ph[:], AF.Tanh,
                            bias=b1p_sb[:, dout : dout + 1], scale=1.0,
                        )

            emit_a(0)
            emit_a(1)
            emit_a(2)
            emit_b(0)
            emit_natural_chain(0)
            emit_a(3)
            emit_b(1)
            emit_natural_chain(1)
            emit_b(2)
            emit_natural_chain(2)
            emit_b(3)

            # ---- freq-major phases for the own chunk (csc [64, CH]) ----
            ttF = cp_.tile([64, CH], BF16)
            argF = cp_.tile([64, CH], F32)
            sinF = cp_.tile([64, CH], F32)
            csc = cp_.tile([64, CH], BF16)
            nc.gpsimd.tensor_copy(ttF[0:32, :], tt_sb[96:128, :])
            nc.gpsimd.tensor_copy(ttF[32:64, :], tt_sb[96:128, :])
            nc.vector.scalar_tensor_tensor(
                out=argF[:], in0=ttF[:], scalar=kc_val, in1=tblF_sb[:],
                op0=ALU.mult, op1=ALU.add,
            )
            # one tanh->sin table switch, then every sin in a row. The
            # zero-valued bias token depends on ALL (b) tanh rows, which
            # pins the sins after the last tanh in the schedule.
            tok = cp_.tile([128, 1], F32)
            nc.vector.tensor_scalar_mul(tok[:], tt_sb[:, 0:1], 0.0)
            for c in range(3):
                nc.scalar.activation(
                    sinN2[:, c], argN2[:, c], AF.Sin, bias=tok[:]
                )
                nc.gpsimd.tensor_mul(csm2N[:, c], sinN2[:, c], sgnN_sb[:, c])
            nc.scalar.activation(sinF[:], argF[:], AF.Sin, bias=tok[0:64, :])
            nc.gpsimd.tensor_mul(csc[:], sinF[:], sgnF_sb[:])
            # preload the sqrt table after the last sin (gated by a token on
            # the final sin output); only Copy/Sqrt remain on scalar after
            # this, so the load hides under the retrieve matmuls
            tok2 = cp_.tile([128, 1], F32)
            nc.vector.tensor_scalar_mul(tok2[:], sinN2[:, 2, 0, 0, 0:1], 0.0)
            dsq = cp_.tile([128, 4], F32)
            nc.scalar.activation(dsq[:], onesf[:, 0:4], AF.Sqrt, bias=tok2[:])

            # PE: value fills while the sin batch runs, then pg + intra
            emit_value_mm(0)
            emit_value_mm(1)
            emit_value_mm(2)
            emit_value_mm(3)
            emit_value_copy(0, "v")
            emit_pg(0)
            emit_pg(1)
            emit_pg(2)
            emit_value_copy(1, "s")

            # ---- (e) intra-chunk scores, triu-masked (own PSUM pool so the
            # value copies never gate them) ----
            p_sb = cp_.tile([128, 4, CH], BF16)
            for tr in range(4):
                psc = pintra.tile([128, CH], F32, tag="pintra")
                nc.tensor.matmul(
                    psc[:], csc[:, ts(tr, 128)], csc[:],
                    start=True, stop=True,
                )
                nc.vector.tensor_mul(p_sb[:, tr, :], psc[:], tri[:, tr, :])
            emit_value_copy(2, "v")
            emit_value_copy(3, "s")

            g_sb = cp_.tile([64, D], BF16)
            nc.vector.tensor_copy(g_sb[:], pg[:])
            gt_sb = cp_.tile([128, 4, 64], BF16)
            ptg = ptrp.tile([128, 4, 64], BF16, tag="ptr", name="ptg")
            for kk in range(4):
                nc.tensor.matmul(
                    ptg[:, kk, :], g_sb[:, ts(kk, 128)], identb[0:64, 0:64],
                    is_transpose=True, skip_group_check=True,
                )
            nc.vector.tensor_copy(gt_sb[:], ptg[:])
            pst = pone.tile([64, D], F32, tag="pst")
            for kk in range(4):
                nc.tensor.matmul(
                    pst[:], gt_sb[:, kk, :], wv_sb[:, kk, :],
                    start=(kk == 0), stop=(zero_bv and kk == 3),
                )
            if not zero_bv:
                # msum[j] = sum_l CS[l, j]; state += msum (x) bv
                pms = ptrp.tile([64, 1], F32, tag="ptr", name="pms")
                first = True
                for c in range(3):
                    for bb in range(4):
                        nc.tensor.matmul(
                            pms[:], csm2N[:, c, bb, :, :], onesc[:],
                            start=first, stop=(c == 2 and bb == 3),
                        )
                        first = False
                ms_sb = cp_.tile([64, 1], BF16)
                nc.vector.tensor_copy(ms_sb[:], pms[:])
                msT = cp_.tile([1, 64], BF16)
                ptm = ptrp.tile([128, 128], BF16, tag="ptr", name="ptm")
                nc.tensor.transpose(
                    ptm[0:1, 0:64], ms_sb[:], identb[0:64, 0:64]
                )
                nc.vector.tensor_copy(msT[:], ptm[0:1, 0:64])
                nc.tensor.matmul(pst[:], msT[:], bvr_sb[:], start=False, stop=True)
            state_sb = cp_.tile([64, D], BF16)
            nc.vector.tensor_copy(state_sb[:], pst[:])

            # ---- (f) retrieved^T [D, CH], stats interleaved ----
            retrT = cp_.tile([128, 4, CH], BF16)
            sq_sb = cp_.tile([128, 4, CH], BF16)
            ps_mean = pone.tile([1, CH], F32, tag="pst")
            ps_sq = pintra.tile([1, CH], F32, tag="pintra")

            def emit_retr(dd):
                pr = pmm.tile([128, CH], F32, tag="pmm")
                for tr in range(4):
                    nc.tensor.matmul(
                        pr[:], value_sb[:, tr, ts(dd, 128)], p_sb[:, tr, :],
                        start=(tr == 0), stop=False,
                    )
                nc.tensor.matmul(
                    pr[:], state_sb[:, ts(dd, 128)], csc[:],
                    start=False, stop=True,
                )
                if dd % 2 == 0:
                    nc.scalar.copy(retrT[:, dd, :], pr[:])
                else:
                    nc.vector.tensor_copy(retrT[:, dd, :], pr[:])
                nc.vector.tensor_mul(
                    sq_sb[:, dd, :], retrT[:, dd, :], retrT[:, dd, :]
                )

            def emit_stat(dd):
                nc.tensor.matmul(
                    ps_mean[0:1, :], onesc[:], retrT[:, dd, :],
                    start=(dd == 0), stop=(dd == 3),
                )
                nc.tensor.matmul(
                    ps_sq[0:1, :], onesc[:], sq_sb[:, dd, :],
                    start=(dd == 0), stop=(dd == 3),
                )

            emit_retr(0)
            emit_retr(1)
            emit_stat(0)
            emit_retr(2)
            emit_stat(1)
            emit_retr(3)
            emit_stat(2)
            emit_stat(3)

            # ---- LayerNorm rstd: bounce raw sums through DRAM into a
            # column layout [128, 8], then tiny per-partition math ----
            mu_n = cp_.tile([1, CH], BF16)
            nc.vector.tensor_scalar_mul(mu_n[:], ps_mean[0:1, :], 1.0 / D)
            stat_row = cp_.tile([1, 2 * CH], F32)
            nc.vector.tensor_copy(stat_row[0:1, 0:CH], ps_mean[0:1, :])
            nc.scalar.copy(stat_row[0:1, CH:], ps_sq[0:1, :])
            pstT = ptrp.tile([128, 8], F32, tag="ptr", name="pstT")
            for q in range(8):
                nc.tensor.matmul(
                    pstT[:, q : q + 1], stat_row[0:1, ts(q, 128)],
                    onesf[0:1, 0:1], is_transpose=True, skip_group_check=True,
                )
            statsT = cp_.tile([128, 8], F32)
            nc.vector.tensor_copy(statsT[:], pstT[:])
            muT = cp_.tile([128, 4], F32)
            nc.vector.tensor_scalar_mul(muT[:], statsT[:, 0:4], 1.0 / D)
            varT = cp_.tile([128, 4], F32)
            nc.vector.tensor_scalar_mul(varT[:], statsT[:, 4:8], 1.0 / D)
            mu2T = cp_.tile([128, 4], F32)
            nc.vector.tensor_mul(mu2T[:], muT[:], muT[:])
            nc.vector.tensor_sub(varT[:], varT[:], mu2T[:])
            nc.vector.tensor_add(varT[:], varT[:], epsn_sb[:])
            sdT = cp_.tile([128, 4], F32)
            nc.scalar.activation(sdT[:], varT[:], AF.Sqrt)
            rstdT = cp_.tile([128, 4], F32)
            nc.vector.reciprocal(rstdT[:], sdT[:])

            # ---- (h) out = rstd*(retr^T @ Wo' + mu*ncs) + res ----
            for tt in range(4):
                pho = pmm.tile([128, D], F32, tag="pmm")
                for ee in range(4):
                    nc.tensor.matmul(
                        pho[:], retrT[:, ee, ts(tt, 128)], wo_sb[:, ee, :],
                        start=(ee == 0), stop=False,
                    )
                nc.tensor.matmul(
                    pho[:], mu_n[0:1, ts(tt, 128)], ncs_sb[:],
                    start=False, stop=True,
                )
                out_t = rot4.tile([128, D], BF16, tag="outt")
                if tt % 2 == 0:
                    nc.vector.scalar_tensor_tensor(
                        out=out_t[:], in0=pho[:], scalar=rstdT[:, tt : tt + 1],
                        in1=res_sb[:, tt, :], op0=ALU.mult, op1=ALU.add,
                    )
                else:
                    tmp_t = rot4.tile([128, D], F32, tag="tmpt")
                    nc.scalar.mul(tmp_t[:], pho[:], rstdT[:, tt : tt + 1])
                    nc.vector.tensor_add(out_t[:], tmp_t[:], res_sb[:, tt, :])
                nc.sync.dma_start(y[ts(tt, 128), :], out_t[:])

    nc.compile()
    return nc


def _get_nc(zero_bv: bool, kc_val: float):
    key = ("nc", zero_bv, round(kc_val, 9))
    if key not in _NC_CACHE:
        _NC_CACHE[key] = _build(zero_bv, kc_val)
    return _NC_CACHE[key]


def _prep_inputs(inputs):
    x = np.asarray(inputs["x"], np.float32)
    W1 = np.asarray(inputs["W1"], np.float32)
    b1 = np.asarray(inputs["b1"], np.float32)
    W2 = np.asarray(inputs["W2"], np.float32)
    b2 = np.asarray(inputs["b2"], np.float32)
    pos_scale = float(np.asarray(inputs["pos_scale"]).reshape(-1)[0])
    content_scale = float(np.asarray(inputs["content_scale"]).reshape(-1)[0])
    Wv = np.asarray(inputs["Wv"], np.float32)
    bv = np.asarray(inputs["bv"], np.float32)
    ln_g = np.asarray(inputs["ln_g"], np.float32)
    ln_b = np.asarray(inputs["ln_b"], np.float32)
    Wo = np.asarray(inputs["Wo"], np.float32)
    bo = np.asarray(inputs["bo"], np.float32)

    bf16 = ml_dtypes.bfloat16
    Wop = ln_g[:, None] * Wo                       # fold ln gain
    ncs_v = -Wop.sum(axis=0, dtype=np.float64).astype(np.float32)[None, :]
    res_base = (ln_b @ Wo + bo).astype(np.float32)  # fold ln bias + out bias

    # [p, k, out]: row Din = 128k+p  (exact SBUF layout, contiguous DMA)
    w1_t = np.ascontiguousarray(W1.reshape(4, 128, D).transpose(1, 0, 2))
    w2_t = np.ascontiguousarray(W2.reshape(4, 128, K).transpose(1, 0, 2))
    wv_t = np.ascontiguousarray(Wv.reshape(4, 128, D).transpose(1, 0, 2))
    wo_t = np.ascontiguousarray(Wop.reshape(4, 128, D).transpose(1, 0, 2))
    b1p = np.ascontiguousarray(b1.reshape(4, 128).T)
    b2p = np.tile(b2, 4)[:, None].astype(np.float32)
    kc = np.full((128, 1), np.pi * content_scale, np.float32)
    bvr = bv[None, :].astype(np.float32)

    freqs = 1.0 / (10000.0 ** (np.arange(K, dtype=np.float64) / K))

    def packN(a):
        # [T, K] -> [128p, 4c, 4b, 32f]: natural row l = 512c + 128b + p
        t = a.reshape(4, 4, 128, K)  # [c, b, p, f]
        return t.transpose(2, 0, 1, 3)

    in_maps = []
    for core in range(NCORE):
        b, i = divmod(core, 4)
        pad = 1536 - 512 * i
        nreal = 512 * (i + 1)
        xpad = np.zeros((T, D), np.float32)
        xpad[pad:] = x[b, :nreal]
        # xt dram layout: [c, 128, 4, CH]: [p, k] = Din 128k+p, per-chunk contiguous
        xt = np.ascontiguousarray(
            xpad.T.reshape(4, 128, 4, CH).transpose(2, 1, 0, 3))

        lidx = np.arange(T, dtype=np.float64) - pad
        ang = pos_scale * lidx[:, None] * freqs[None, :]      # [T, K]
        # S path: sin(ang + ct) -> fold ang = ps + pi*n, ps in [-pi/2, pi/2]
        n_s = np.round(ang / np.pi)
        ps_f = (ang - np.pi * n_s).astype(np.float32)
        sg_s = np.where(n_s % 2 == 0, 1.0, -1.0).astype(np.float32)
        # C path: cos(ang + ct) = sin(pi/2 + ang + ct)
        n_c = np.round((ang + np.pi / 2) / np.pi)
        pc_f = (ang + np.pi / 2 - np.pi * n_c).astype(np.float32)
        sg_c = np.where(n_c % 2 == 0, 1.0, -1.0).astype(np.float32)
        # padded rows contribute nothing: zero the signs (C = S = 0)
        sg_s[lidx < 0] = 0.0
        sg_c[lidx < 0] = 0.0
        ps_f[lidx < 0] = 0.0
        pc_f[lidx < 0] = 0.0

        # [128, 3, 4, 2, 32]: prefix chunks only, (path, f) innermost
        tblN_a = np.stack([packN(pc_f), packN(ps_f)], axis=3)[:, 0:3]
        sgnN_a = np.stack([packN(sg_c), packN(sg_s)], axis=3)[:, 0:3]
        # own chunk, freq-major [path*32+f, t]
        tblF_a = np.concatenate([pc_f[1536:].T, ps_f[1536:].T], axis=0)
        sgnF_a = np.concatenate([sg_c[1536:].T, sg_s[1536:].T], axis=0)
        epsn_r = (1e-5 * (np.arange(512 * i + 1, 512 * i + CH + 1,
                                    dtype=np.float64) * K)).astype(np.float32)
        epsn_a = np.ascontiguousarray(epsn_r.reshape(4, 128).T)  # [128p, 4tt]

        resc = (x[b, 512 * i : 512 * i + CH] + res_base[None, :]).astype(np.float32)

        xnat = np.ascontiguousarray(
            xpad[0:1536].reshape(12, 128, D).transpose(1, 0, 2))
        in_maps.append({
            "xt": xt.astype(bf16), "xn": xnat.astype(bf16),
            "w1f": np.ascontiguousarray(w1_t[:, 0:1, :].astype(bf16)),
            "w1": w1_t.astype(bf16), "w2": w2_t.astype(bf16),
            "wv": wv_t.astype(bf16), "wo": wo_t.astype(bf16),
            "ncs": ncs_v.astype(bf16), "bvr": bvr.astype(bf16),
            "b1p": b1p, "b2p": b2p, "kc": kc,
            "tblN": np.ascontiguousarray(tblN_a.astype(np.float32)),
            "sgnN": np.ascontiguousarray(sgnN_a.astype(bf16)),
            "tblF": np.ascontiguousarray(tblF_a.astype(np.float32)),
            "sgnF": np.ascontiguousarray(sgnF_a.astype(bf16)),
            "epsn": epsn_a,
            "res": np.ascontiguousarray(
                resc.reshape(4, 128, D).transpose(1, 0, 2)).astype(bf16),
        })
    return in_maps


def kernel(**inputs) -> np.ndarray:
    global LAST_RESULT
    zero_bv = bool(np.all(np.asarray(inputs["bv"]) == 0.0))
    kc_val = float(np.pi * float(np.asarray(inputs["content_scale"]).reshape(-1)[0]))
    nc = _get_nc(zero_bv, kc_val)
    in_maps = _prep_inputs(inputs)
    result = run_bass_kernel_spmd(
        nc, in_maps, core_ids=list(range(NCORE)), **RUN_KWARGS
    )
    LAST_RESULT = result
    y = np.empty((B, L, D), np.float32)
    for core in range(NCORE):
        b, i = divmod(core, 4)
        y[b, 512 * i : 512 * i + CH] = np.asarray(
            result.results[core]["y"], dtype=np.float32)
    return y

